# revision 18
# baseline (speedup 1.0000x reference)
"""CMLANet Trainium2 kernel: 8-core SPMD, time-sharded, bf16 DEER.

Main GRU and the two small attention GRUs run scan-accelerated
fixed-point (DEER) iterations: gates from the previous iterate via bf16
PE matmuls, then the linear recurrence h_t = z_t h_{t-1} + (1-z_t) n_t
applied exactly with the fp32 hardware prefix scan.  Time is sharded
over 8 cores (256 steps each) with short halos; core 0's halo is pinned
to the true zero initial state by masking the scan inputs.  The two
small GRUs (a/o) are packed into 80 partitions with block-structured
weights.  The bilinear U@m1 products are topic-sharded across cores
(10 of 80 (u,k) pairs each) and all-gathered; softmax normalization is
folded into one AllReduce via the unnormalized-exp trick.
"""

import os
import sys
import numpy as np
import ml_dtypes

sys.path.insert(0, "/opt/trn_rl_repo")

import concourse.bass as bass  # noqa: E402,F401
import concourse.bacc as bacc  # noqa: E402
import concourse.tile as tile  # noqa: E402
from concourse import mybir  # noqa: E402
from concourse.bass_utils import run_bass_kernel_spmd  # noqa: E402

F32 = mybir.dt.float32
BF16 = mybir.dt.bfloat16
ALU = mybir.AluOpType
AF = mybir.ActivationFunctionType
AX = mybir.AxisListType

BS, T, CS, DE, NH, NT, NC = 4, 2048, 5, 300, 512, 20, 5
NV = 2 * NT          # 40
NP = 2 * NV          # 80 packed (s=0, s=1)
NIN = DE * CS        # 1500
KIN = 12
G = 3 * NH
NGC = 12
NJC = 4
NCORE = 8
CHUNK = T // NCORE   # 256
HALO_M = 64
TM = CHUNK + HALO_M  # 320
HALO_S = 48
TS = CHUNK + HALO_S  # 304
ITERS_MAIN = int(os.environ.get("CMLA_ITERS_MAIN", "7"))
ITERS_SMALL = int(os.environ.get("CMLA_ITERS_SMALL", "5"))
PSH = 10             # (u,k) pairs per core

TCS = [(0, 128), (128, 128), (256, 64)]
STCS = [(0, 128), (128, 128), (256, 48)]

_CACHE = {}


def build_program(debug=False):
    nc = bacc.Bacc("TRN2", target_bir_lowering=False, debug=False,
                   num_devices=NCORE)

    def din(name, shape, dt=F32):
        return nc.dram_tensor(name, list(shape), dt,
                              kind="ExternalInput").ap()

    d = {}
    d["xT"] = din("xT", [KIN, 128, BS, TM], BF16)
    d["wihT"] = din("wihT", [KIN, 128, NGC, 128], BF16)
    d["whhT"] = din("whhT", [NJC, 128, NGC, 128], BF16)
    d["biasc"] = din("biasc", [128, NGC])
    d["bhhn"] = din("bhhn", [128, NJC])
    d["hmaskM"] = din("hmaskM", [128, BS, 128])
    d["i128"] = din("i128", [128, 128], BF16)
    d["um0p"] = din("um0p", [128, NJC, NP], BF16)
    d["i80"] = din("i80", [NP, NP], BF16)
    d["gwih2"] = din("gwih2", [NP, 3, NP], BF16)
    d["gwhh2"] = din("gwhh2", [NP, 3, NP], BF16)
    d["biasr2"] = din("biasr2", [NP, 1])
    d["biasz2"] = din("biasz2", [NP, 1])
    d["biasn2"] = din("biasn2", [NP, 1])
    d["bhhn2"] = din("bhhn2", [NP, 1])
    d["smaskS"] = din("smaskS", [NP, BS, 128])
    d["vavo2"] = din("vavo2", [NP, 2], BF16)
    d["tmloc"] = din("tmloc", [2, BS, CHUNK])
    d["tmv32"] = din("tmv32", [128, 32])
    d["ustat"] = din("ustat", [PSH, NJC, 128, NJC, 128], BF16)
    d["laloT2"] = din("laloT2", [NP, 10], BF16)
    d["lalob2"] = din("lalob2", [10, 1])
    d["onesg"] = din("onesg", [10, 2], BF16)

    d["y"] = nc.dram_tensor("y", [10, BS, CHUNK], F32,
                            kind="ExternalOutput").ap()
    if debug:
        d["hdbg"] = nc.dram_tensor("hdbg", [NJC, 128, BS, TM], F32,
                                   kind="ExternalOutput").ap()
        d["r1dbg"] = nc.dram_tensor("r1dbg", [NP, BS, TS], F32,
                                    kind="ExternalOutput").ap()
        d["m1dbg"] = nc.dram_tensor("m1dbg", [128, 32], F32,
                                    kind="ExternalOutput").ap()

    d["cc2_in"] = nc.dram_tensor("cc2_in", [128, 36], F32)
    d["cc2_out"] = nc.dram_tensor("cc2_out", [128, 36], F32,
                                  addr_space="Shared")
    d["cc3_in"] = nc.dram_tensor("cc3_in", [128, PSH * 16], F32)
    d["cc3_out"] = nc.dram_tensor("cc3_out", [NCORE, 128, PSH * 16], F32,
                                  addr_space="Shared")

    with tile.TileContext(nc, num_cores=NCORE) as tc:
        _body(nc, tc, d, debug)
    nc.compile()
    return nc


def _body(nc, tc, d, debug):
    from contextlib import ExitStack
    es = ExitStack()
    ex = es.enter_context(tc.tile_pool(name="persist", bufs=1))

    # persistent state: f32 scan state + bf16 matmul copy
    Hf = [ex.tile([128, BS, TM + 1], F32, name=f"Hf{j}") for j in range(NJC)]
    Hb = [ex.tile([128, BS, TM + 1], BF16, name=f"Hb{j}")
          for j in range(NJC)]
    for j in range(NJC):
        nc.vector.memset(Hf[j][:, :, :], 0.0)
        nc.vector.memset(Hb[j][:, :, :], 0.0)

    # ======================= main GRU =======================
    with tc.tile_pool(name="xwpool", bufs=1) as xp:
        xwr = [xp.tile([128, BS, TM], BF16, name=f"xwr{g}")
               for g in range(8)]          # r: 0-3, z: 4-7 (bias folded)
        xwn = [xp.tile([128, BS, TM], F32, name=f"xwn{j}")
               for j in range(NJC)]
        i128 = xp.tile([128, 128], BF16, name="i128t")
        biasc = xp.tile([128, NGC], F32, name="biasct")
        bhhn = xp.tile([128, NJC], F32, name="bhhnt")
        hmaskM = xp.tile([128, BS, 128], F32, name="hmaskMt")
        nc.sync.dma_start(i128[:, :], d["i128"])
        nc.sync.dma_start(biasc[:, :], d["biasc"])
        nc.sync.dma_start(bhhn[:, :], d["bhhn"])
        nc.sync.dma_start(hmaskM[:, :, :], d["hmaskM"])

        # ---- xw = x @ Wih^T (+bih, +bhh for r,z) ----
        with tc.tile_pool(name="xstream", bufs=2) as st, \
                tc.tile_pool(name="wihpool", bufs=1) as wp, \
                tc.tile_pool(name="ps_xw", bufs=3, space="PSUM") as ps:
            wih = wp.tile([128, KIN, NGC, 128], BF16, name="wih")
            nc.sync.dma_start(wih[:, :, :, :],
                              d["wihT"].rearrange("k q g p -> q k g p"))
            for (t0, tw) in TCS:
                xs = st.tile([128, KIN, BS, tw], BF16, name="xs", tag="xs")
                for kc in range(KIN):
                    nc.sync.dma_start(xs[:, kc, :, :],
                                      d["xT"][kc, :, :, t0:t0 + tw])
                for g in range(NGC):
                    p = ps.tile([128, BS, tw], F32, name="xwp", tag="xwp")
                    for kc in range(KIN):
                        nc.tensor.matmul(p[:, :, :], wih[:, kc, g, :],
                                         xs[:, kc, :, :],
                                         start=(kc == 0),
                                         stop=(kc == KIN - 1))
                    o = xwr[g] if g < 8 else xwn[g - 8]
                    nc.scalar.activation(o[:, :, t0:t0 + tw], p[:, :, :],
                                         AF.Identity,
                                         bias=biasc[:, g:g + 1])

        # ---- DEER iterations ----
        with tc.tile_pool(name="sc_deer", bufs=2) as sc, \
                tc.tile_pool(name="zbpool", bufs=1) as zb, \
                tc.tile_pool(name="ps_deer", bufs=2, space="PSUM") as ps:
            whh = sc.tile([128, NJC, NGC, 128], BF16, name="whh")
            nc.sync.dma_start(whh[:, :, :, :],
                              d["whhT"].rearrange("j q g p -> q j g p"))
            zcw = [zb.tile([128, BS, TM], F32, name=f"zcw{j}")
                   for j in range(NJC)]
            bcw = [zb.tile([128, BS, TM], F32, name=f"bcw{j}")
                   for j in range(NJC)]

            def gate_tail(j, t0, tw, r_src, z_src, n_src):
                # r/z activations, n-path, bm into bcw (masked on chunk 0)
                r = sc.tile([128, BS, tw], F32, name="r", tag="r")
                nc.scalar.activation(r[:, :, :], r_src, AF.Sigmoid)
                nc.scalar.activation(zcw[j][:, :, t0:t0 + tw], z_src,
                                     AF.Sigmoid)
                npre = sc.tile([128, BS, tw], F32, name="npre", tag="npre")
                if n_src is None:
                    # it0: pn = 0 -> npre = r*bhhn + xwn
                    nc.vector.scalar_tensor_tensor(
                        npre[:, :, :], r[:, :, :], bhhn[:, j:j + 1],
                        xwn[j][:, :, t0:t0 + tw], ALU.mult, ALU.add)
                else:
                    rhn = sc.tile([128, BS, tw], F32, name="rhn",
                                  tag="rhn")
                    nc.vector.scalar_tensor_tensor(
                        rhn[:, :, :], n_src, bhhn[:, j:j + 1], r[:, :, :],
                        ALU.add, ALU.mult)
                    nc.vector.tensor_add(npre[:, :, :], rhn[:, :, :],
                                         xwn[j][:, :, t0:t0 + tw])
                nt_ = sc.tile([128, BS, tw], F32, name="nt", tag="nt")
                nc.scalar.activation(nt_[:, :, :], npre[:, :, :], AF.Tanh)
                if t0 == 0:
                    bm = sc.tile([128, BS, tw], F32, name="bm", tag="bm")
                    nc.vector.scalar_tensor_tensor(
                        bm[:, :, :], zcw[j][:, :, t0:t0 + tw], 1.0,
                        nt_[:, :, :], ALU.subtract, ALU.mult)
                    nc.vector.tensor_mul(bcw[j][:, :, t0:t0 + tw],
                                         bm[:, :, :], hmaskM[:, :, :])
                else:
                    nc.vector.scalar_tensor_tensor(
                        bcw[j][:, :, t0:t0 + tw],
                        zcw[j][:, :, t0:t0 + tw], 1.0, nt_[:, :, :],
                        ALU.subtract, ALU.mult)

            def scan_and_cast():
                for j in range(NJC):
                    for b in range(BS):
                        nc.vector.tensor_tensor_scan(
                            Hf[j][:, b, 1:TM + 1], zcw[j][:, b, :],
                            bcw[j][:, b, :], Hf[j][:, b, 0:1],
                            ALU.mult, ALU.subtract)
                for j in range(NJC):
                    nc.scalar.copy(Hb[j][:, :, :], Hf[j][:, :, :])

            # iteration 0: H = 0, no matmuls
            for (t0, tw) in TCS:
                for j in range(NJC):
                    gate_tail(j, t0, tw, xwr[j][:, :, t0:t0 + tw],
                              xwr[4 + j][:, :, t0:t0 + tw], None)
            scan_and_cast()

            for it in range(1, ITERS_MAIN):
                for (t0, tw) in TCS:
                    for j in range(NJC):
                        pr = ps.tile([128, BS, tw], F32, name="pr",
                                     tag="pr")
                        pz = ps.tile([128, BS, tw], F32, name="pz",
                                     tag="pz")
                        pn = ps.tile([128, BS, tw], F32, name="pn",
                                     tag="pn")
                        nc.tensor.matmul(pr[:, :, :], i128[:, :],
                                         xwr[j][:, :, t0:t0 + tw],
                                         start=True, stop=False)
                        nc.tensor.matmul(pz[:, :, :], i128[:, :],
                                         xwr[4 + j][:, :, t0:t0 + tw],
                                         start=True, stop=False)
                        for jc in range(NJC):
                            hs = Hb[jc][:, :, t0:t0 + tw]
                            nc.tensor.matmul(pr[:, :, :],
                                             whh[:, jc, j, :], hs,
                                             start=False,
                                             stop=(jc == NJC - 1))
                            nc.tensor.matmul(pz[:, :, :],
                                             whh[:, jc, 4 + j, :], hs,
                                             start=False,
                                             stop=(jc == NJC - 1))
                            nc.tensor.matmul(pn[:, :, :],
                                             whh[:, jc, 8 + j, :], hs,
                                             start=(jc == 0),
                                             stop=(jc == NJC - 1))
                        gate_tail(j, t0, tw, pr[:, :, :], pz[:, :, :],
                                  pn[:, :, :])
                scan_and_cast()

        if debug:
            for j in range(NJC):
                nc.sync.dma_start(d["hdbg"][j], Hf[j][:, :, 1:TM + 1])

    # ================== attention + small GRUs ==================
    ex2 = es.enter_context(tc.tile_pool(name="persist2", bufs=1))
    us = ex2.tile([128, PSH, NJC, NJC, 128], BF16, name="us")
    nc.sync.dma_start(us[:, :, :, :, :],
                      d["ustat"].rearrange("i jc q hc p -> q i jc hc p"))
    um0p = ex2.tile([128, NJC, NP], BF16, name="um0pt")
    i80 = ex2.tile([NP, NP], BF16, name="i80t")
    gwih2 = ex2.tile([NP, 3, NP], BF16, name="gwih2t")
    gwhh2 = ex2.tile([NP, 3, NP], BF16, name="gwhh2t")
    biasr2 = ex2.tile([NP, 1], F32, name="biasr2t")
    biasz2 = ex2.tile([NP, 1], F32, name="biasz2t")
    biasn2 = ex2.tile([NP, 1], F32, name="biasn2t")
    bhhn2 = ex2.tile([NP, 1], F32, name="bhhn2t")
    smaskS = ex2.tile([NP, BS, 128], F32, name="smaskSt")
    vavo2 = ex2.tile([NP, 2], BF16, name="vavo2t")
    tmloc = ex2.tile([2, BS, CHUNK], F32, name="tmloct")
    tmv32 = ex2.tile([128, 32], F32, name="tmv32t")
    for name, t in (("um0p", um0p), ("i80", i80), ("gwih2", gwih2),
                    ("gwhh2", gwhh2), ("biasr2", biasr2),
                    ("biasz2", biasz2), ("biasn2", biasn2),
                    ("bhhn2", bhhn2), ("smaskS", smaskS),
                    ("vavo2", vavo2), ("tmloc", tmloc),
                    ("tmv32", tmv32)):
        if len(t.shape) == 2:
            nc.sync.dma_start(t[:, :], d[name])
        else:
            nc.sync.dma_start(t[:, :, :], d[name])

    def tp_gates(lhs_of, sxw_r, sxw_z, sxw_n, perb):
        """a = tanh(sum_hc lhsT_hc @ Hb_hc); small-gate xw tiles."""
        with tc.tile_pool(name="sc_tp", bufs=2) as sc, \
                tc.tile_pool(name="ps_tp", bufs=2, space="PSUM") as ps:
            for (q0, qw) in STCS:
                a = sc.tile([NP, BS, qw], BF16, name="a", tag="a")
                if perb:
                    for b in range(BS):
                        pa = ps.tile([NP, qw], F32, name="pa", tag="pa")
                        for hc in range(NJC):
                            nc.tensor.matmul(
                                pa[:, :], lhs_of(hc, b),
                                Hb[hc][:, b, 17 + q0:17 + q0 + qw],
                                start=(hc == 0), stop=(hc == NJC - 1))
                        nc.scalar.activation(a[:, b, :], pa[:, :],
                                             AF.Tanh)
                else:
                    pa = ps.tile([NP, BS, qw], F32, name="pab",
                                 tag="pab")
                    for hc in range(NJC):
                        nc.tensor.matmul(
                            pa[:, :, :], lhs_of(hc, None),
                            Hb[hc][:, :, 17 + q0:17 + q0 + qw],
                            start=(hc == 0), stop=(hc == NJC - 1))
                    nc.scalar.activation(a[:, :, :], pa[:, :, :],
                                         AF.Tanh)
                for gi, (xwg, bsl) in enumerate(
                        ((sxw_r, biasr2), (sxw_z, biasz2),
                         (sxw_n, biasn2))):
                    px = ps.tile([NP, BS, qw], F32, name="px", tag="px")
                    nc.tensor.matmul(px[:, :, :], gwih2[:, gi, :],
                                     a[:, :, :], start=True, stop=True)
                    nc.scalar.activation(xwg[:, :, q0:q0 + qw],
                                         px[:, :, :], AF.Identity,
                                         bias=bsl[:, 0:1])

    def small_deer(sxw_r, sxw_z, sxw_n, Rf, Rb):
        with tc.tile_pool(name="sc_sd", bufs=2) as sc, \
                tc.tile_pool(name="zbs", bufs=1) as zbs, \
                tc.tile_pool(name="ps_sd", bufs=2, space="PSUM") as ps:
            zcs = zbs.tile([NP, BS, TS], F32, name="zcs")
            bcs = zbs.tile([NP, BS, TS], F32, name="bcs")

            def tail(q0, qw, r_src, z_src, n_src):
                r = sc.tile([NP, BS, qw], F32, name="sr", tag="sr")
                nc.scalar.activation(r[:, :, :], r_src, AF.Sigmoid)
                nc.scalar.activation(zcs[:, :, q0:q0 + qw], z_src,
                                     AF.Sigmoid)
                npre = sc.tile([NP, BS, qw], F32, name="snpre",
                               tag="snpre")
                if n_src is None:
                    nc.vector.scalar_tensor_tensor(
                        npre[:, :, :], r[:, :, :], bhhn2[:, 0:1],
                        sxw_n[:, :, q0:q0 + qw], ALU.mult, ALU.add)
                else:
                    rhn = sc.tile([NP, BS, qw], F32, name="srhn",
                                  tag="srhn")
                    nc.vector.scalar_tensor_tensor(
                        rhn[:, :, :], n_src, bhhn2[:, 0:1], r[:, :, :],
                        ALU.add, ALU.mult)
                    nc.vector.tensor_add(npre[:, :, :], rhn[:, :, :],
                                         sxw_n[:, :, q0:q0 + qw])
                nt_ = sc.tile([NP, BS, qw], F32, name="snt", tag="snt")
                nc.scalar.activation(nt_[:, :, :], npre[:, :, :],
                                     AF.Tanh)
                if q0 == 0:
                    bm = sc.tile([NP, BS, qw], F32, name="sbm",
                                 tag="sbm")
                    nc.vector.scalar_tensor_tensor(
                        bm[:, :, :], zcs[:, :, q0:q0 + qw], 1.0,
                        nt_[:, :, :], ALU.subtract, ALU.mult)
                    nc.vector.tensor_mul(bcs[:, :, q0:q0 + qw],
                                         bm[:, :, :], smaskS[:, :, :])
                else:
                    nc.vector.scalar_tensor_tensor(
                        bcs[:, :, q0:q0 + qw], zcs[:, :, q0:q0 + qw],
                        1.0, nt_[:, :, :], ALU.subtract, ALU.mult)

            def scan_cast():
                for b in range(BS):
                    nc.vector.tensor_tensor_scan(
                        Rf[:, b, 1:TS + 1], zcs[:, b, :], bcs[:, b, :],
                        Rf[:, b, 0:1], ALU.mult, ALU.subtract)
                nc.scalar.copy(Rb[:, :, :], Rf[:, :, :])

            for (q0, qw) in STCS:
                tail(q0, qw, sxw_r[:, :, q0:q0 + qw],
                     sxw_z[:, :, q0:q0 + qw], None)
            scan_cast()
            for it in range(1, ITERS_SMALL):
                for (q0, qw) in STCS:
                    p_r = ps.tile([NP, BS, qw], F32, name="p_r",
                                  tag="p_r")
                    p_z = ps.tile([NP, BS, qw], F32, name="p_z",
                                  tag="p_z")
                    p_n = ps.tile([NP, BS, qw], F32, name="p_n",
                                  tag="p_n")
                    rs = Rb[:, :, q0:q0 + qw]
                    nc.tensor.matmul(p_r[:, :, :], i80[:, :],
                                     sxw_r[:, :, q0:q0 + qw],
                                     start=True, stop=False)
                    nc.tensor.matmul(p_r[:, :, :], gwhh2[:, 0, :], rs,
                                     start=False, stop=True)
                    nc.tensor.matmul(p_z[:, :, :], i80[:, :],
                                     sxw_z[:, :, q0:q0 + qw],
                                     start=True, stop=False)
                    nc.tensor.matmul(p_z[:, :, :], gwhh2[:, 1, :], rs,
                                     start=False, stop=True)
                    nc.tensor.matmul(p_n[:, :, :], gwhh2[:, 2, :], rs,
                                     start=True, stop=True)
                    tail(q0, qw, p_r[:, :, :], p_z[:, :, :],
                         p_n[:, :, :])
                scan_cast()

    # ---- phase 1 ----
    sxw_r1 = ex2.tile([NP, BS, TS], BF16, name="sxwr1")
    sxw_z1 = ex2.tile([NP, BS, TS], BF16, name="sxwz1")
    sxw_n1 = ex2.tile([NP, BS, TS], F32, name="sxwn1")
    tp_gates(lambda hc, b: um0p[:, hc, :], sxw_r1, sxw_z1, sxw_n1, False)
    Rf1 = ex2.tile([NP, BS, TS + 1], F32, name="Rf1")
    Rb1 = ex2.tile([NP, BS, TS + 1], BF16, name="Rb1")
    nc.vector.memset(Rf1[:, :, :], 0.0)
    nc.vector.memset(Rb1[:, :, :], 0.0)
    small_deer(sxw_r1, sxw_z1, sxw_n1, Rf1, Rb1)
    if debug:
        nc.sync.dma_start(d["r1dbg"], Rf1[:, :, 1:TS + 1])

    # ---- attention middle: unnormalized-exp softmax + context ----
    m1t = ex2.tile([128, 2, BS, NJC], F32, name="m1t")
    with tc.tile_pool(name="sc_att", bufs=1) as sc, \
            tc.tile_pool(name="ps_att", bufs=2, space="PSUM") as ps:
        w = sc.tile([2, BS, CHUNK], F32, name="w")
        for (o0, ow) in ((0, 128), (128, 128)):
            pv = ps.tile([2, BS, ow], F32, name="pv", tag="pv")
            nc.tensor.matmul(pv[:, :, :], vavo2[:, :],
                             Rb1[:, :, 49 + o0:49 + o0 + ow],
                             start=True, stop=True)
            nc.scalar.activation(w[:, :, o0:o0 + ow], pv[:, :, :],
                                 AF.Exp)
        nc.vector.tensor_mul(w[:, :, :], w[:, :, :], tmloc[:, :, :])
        cc2t = sc.tile([128, 36], F32, name="cc2t")
        nc.vector.memset(cc2t[:, :], 0.0)
        for b in range(BS):
            nc.vector.tensor_reduce(cc2t[0:2, 32 + b:32 + b + 1],
                                    w[:, b, :], AX.X, ALU.add)
        for s in range(2):
            for b in range(BS):
                row = sc.tile([1, CHUNK], F32, name="row", tag="row")
                nc.sync.dma_start(row[:, :], w[s:s + 1, b, :])
                wb = sc.tile([128, CHUNK], F32, name="wb", tag="wb")
                nc.gpsimd.partition_broadcast(wb[:, :], row[0:1, :])
                for hc in range(NJC):
                    prod = sc.tile([128, CHUNK], F32, name="prod",
                                   tag="prod")
                    nc.vector.tensor_mul(prod[:, :],
                                         Hf[hc][:, b, 65:TM + 1],
                                         wb[:, :])
                    col = s * 16 + b * 4 + hc
                    nc.vector.tensor_reduce(cc2t[:, col:col + 1],
                                            prod[:, :], AX.X, ALU.add)
        nc.sync.dma_start(d["cc2_in"].ap(), cc2t[:, :])
        nc.gpsimd.collective_compute(
            "AllReduce", ALU.add, replica_groups=[list(range(NCORE))],
            ins=[d["cc2_in"].ap()], outs=[d["cc2_out"].ap()])
        car = sc.tile([128, 36], F32, name="car")
        nc.sync.dma_start(car[:, :], d["cc2_out"].ap())
        rsw = sc.tile([2, BS], F32, name="rsw")
        nc.vector.reciprocal(rsw[:, :], car[0:2, 32:36])
        for s in range(2):
            for b in range(BS):
                rb1 = sc.tile([1, 1], F32, name="rb1", tag="rb1")
                nc.sync.dma_start(rb1[:, :], rsw[s:s + 1, b:b + 1])
                rbb = sc.tile([128, 1], F32, name="rbb", tag="rbb")
                nc.gpsimd.partition_broadcast(rbb[:, :], rb1[0:1, :])
                col = s * 16 + b * 4
                sc_car = sc.tile([128, NJC], F32, name="sc_car",
                                 tag="sc_car")
                nc.vector.tensor_scalar_mul(sc_car[:, :],
                                            car[:, col:col + NJC],
                                            rbb[:, 0:1])
                nc.vector.tensor_add(m1t[:, s, b, :], sc_car[:, :],
                                     tmv32[:, col:col + NJC])
        if debug:
            nc.sync.dma_start(
                d["m1dbg"],
                m1t[:, :, :, :].rearrange("p s b h -> p (s b h)"))

    # ---- um1 = U @ m1, topic-sharded; all-gather ----
    um1g = ex2.tile([128, NCORE, PSH, NJC, BS], BF16, name="um1g")
    m1b = ex2.tile([128, 2, BS, NJC], BF16, name="m1b")
    nc.scalar.copy(m1b[:, :, :, :], m1t[:, :, :, :])
    with tc.tile_pool(name="sc_um", bufs=2) as sc, \
            tc.tile_pool(name="ps_um", bufs=2, space="PSUM") as ps:
        cc3t = sc.tile([128, PSH, NJC, BS], F32, name="cc3t")
        for i in range(PSH):
            m = 0 if i < 5 else 1
            for hc in range(NJC):
                pu = ps.tile([128, BS], F32, name="pu", tag="pu")
                for jc in range(NJC):
                    nc.tensor.matmul(pu[:, :], us[:, i, jc, hc, :],
                                     m1b[:, m, :, jc],
                                     start=(jc == 0),
                                     stop=(jc == NJC - 1))
                nc.scalar.copy(cc3t[:, i, hc, :], pu[:, :])
        nc.sync.dma_start(
            d["cc3_in"].ap(),
            cc3t[:, :, :, :].rearrange("p i h b -> p (i h b)"))
        nc.gpsimd.collective_compute(
            "AllGather", ALU.bypass, replica_groups=[list(range(NCORE))],
            ins=[d["cc3_in"].ap()], outs=[d["cc3_out"].ap()])
        umf = sc.tile([128, NCORE, PSH, NJC, BS], F32, name="umf")
        nc.sync.dma_start(
            umf[:, :, :, :, :],
            d["cc3_out"].ap().rearrange("c q x -> q c x"))
        nc.scalar.copy(um1g[:, :, :, :, :], umf[:, :, :, :, :])

    # ---- phase 2 ----
    sxw_r2 = ex2.tile([NP, BS, TS], BF16, name="sxwr2")
    sxw_z2 = ex2.tile([NP, BS, TS], BF16, name="sxwz2")
    sxw_n2 = ex2.tile([NP, BS, TS], F32, name="sxwn2")
    tp_gates(lambda hc, b: um1g[:, :, :, hc, b], sxw_r2, sxw_z2,
             sxw_n2, True)
    Rf2 = ex2.tile([NP, BS, TS + 1], F32, name="Rf2")
    Rb2 = ex2.tile([NP, BS, TS + 1], BF16, name="Rb2")
    nc.vector.memset(Rf2[:, :, :], 0.0)
    nc.vector.memset(Rb2[:, :, :], 0.0)
    small_deer(sxw_r2, sxw_z2, sxw_n2, Rf2, Rb2)

    # ---- final: ha = R1+R2; logits; per-class softmax ----
    with tc.tile_pool(name="sc_fin", bufs=2) as sc, \
            tc.tile_pool(name="ps_fin", bufs=2, space="PSUM") as ps:
        lal = sc.tile([NP, 10], BF16, name="lal")
        lb = [sc.tile([5, 1], F32, name=f"lb{s}") for s in range(2)]
        ones5 = sc.tile([5, 1], BF16, name="ones5t")
        nc.sync.dma_start(lal[:, :], d["laloT2"])
        for s in range(2):
            nc.sync.dma_start(lb[s][:, :], d["lalob2"][5 * s:5 * s + 5])
        nc.sync.dma_start(ones5[:, :], d["onesg"][0:5, 0:1])
        ha = sc.tile([NP, BS, CHUNK], BF16, name="ha")
        nc.vector.tensor_add(ha[:, :, :], Rf1[:, :, 49:TS + 1],
                             Rf2[:, :, 49:TS + 1])
        yt = [sc.tile([5, BS, CHUNK], F32, name=f"yt{s}")
              for s in range(2)]
        for (o0, ow) in ((0, 128), (128, 128)):
            for s in range(2):
                pl = ps.tile([5, BS, ow], F32, name="pl", tag="pl")
                nc.tensor.matmul(pl[:, :, :], lal[:, 5 * s:5 * s + 5],
                                 ha[:, :, o0:o0 + ow], start=True,
                                 stop=True)
                el = sc.tile([5, BS, ow], F32, name="el", tag="el")
                nc.scalar.activation(el[:, :, :], pl[:, :, :], AF.Exp,
                                     bias=lb[s][:, 0:1])
                elb = sc.tile([5, BS, ow], BF16, name="elb", tag="elb")
                nc.vector.tensor_copy(elb[:, :, :], el[:, :, :])
                pss = ps.tile([1, BS, ow], F32, name="pss", tag="pss")
                nc.tensor.matmul(pss[:, :, :], ones5[:, :],
                                 elb[:, :, :], start=True, stop=True)
                rs = sc.tile([1, BS, ow], F32, name="rs", tag="rs")
                nc.vector.reciprocal(rs[:, :, :], pss[:, :, :])
                rb = sc.tile([5, BS, ow], F32, name="rb", tag="rb")
                nc.gpsimd.partition_broadcast(rb[:, :, :], rs[:, :, :],
                                              channels=5)
                nc.vector.tensor_mul(yt[s][:, :, o0:o0 + ow],
                                     el[:, :, :], rb[:, :, :])
        for s in range(2):
            nc.sync.dma_start(d["y"][5 * s:5 * s + 5], yt[s][:, :, :])
    es.close()


# ----------------------------------------------------------------------------
# host side
# ----------------------------------------------------------------------------

BF = ml_dtypes.bfloat16

# slot -> (u, k) pair assignment: slots 0-4 use m=0 pairs, 5-9 use m=1,
# uniform across cores (required for SPMD).  u: 0=Ua 1=Va 2=Uo 3=Vo.
_M0 = [(0, k) for k in range(NT)] + [(3, k) for k in range(NT)]
_M1 = [(1, k) for k in range(NT)] + [(2, k) for k in range(NT)]
PAIRS = []
for _c in range(NCORE):
    PAIRS += _M0[_c * 5:(_c + 1) * 5] + _M1[_c * 5:(_c + 1) * 5]


def _chan(u, k):
    """Canonical packed-80 a-channel for pair (u, k)."""
    s = 0 if u < 2 else 1
    within = k if u in (0, 2) else NT + k
    return s * NV + within


def _prep_inputs(inputs):
    inp = {k: (np.asarray(v) if not np.isscalar(v) else v)
           for k, v in inputs.items()}
    emb = np.asarray(inp["emb"], np.float32)
    idx = np.asarray(inp["index_embed"])
    cw = np.asarray(inp["context_words"])
    seq = int(np.asarray(inp["seq_size"]))

    tok = emb.T[idx]
    pad = np.broadcast_to(np.asarray(inp["padding"], np.float32),
                          (BS, 1, DE))
    pkt = np.broadcast_to(np.asarray(inp["punkt"], np.float32),
                          (BS, 1, DE))
    nodes = np.concatenate([tok, pad, pkt], axis=1).astype(np.float32)
    x = np.stack([nodes[b][cw[b]] for b in range(BS)]).reshape(BS, T, NIN)
    xpad = np.zeros((BS, T, KIN * 128), np.float32)
    xpad[:, :, :NIN] = x

    Wih = np.asarray(inp["gru_Wih"], np.float32)
    Whh = np.asarray(inp["gru_Whh"], np.float32)
    bih = np.asarray(inp["gru_bih"], np.float32)
    bhh = np.asarray(inp["gru_bhh"], np.float32)

    wpd = np.zeros((G, KIN * 128), np.float32)
    wpd[:, :NIN] = Wih
    wihT = np.ascontiguousarray(
        wpd.reshape(NGC, 128, KIN, 128).transpose(2, 3, 0, 1)).astype(BF)
    whhT = np.ascontiguousarray(
        Whh.reshape(NGC, 128, NJC, 128).transpose(2, 3, 0, 1)).astype(BF)
    biasc = np.zeros((128, NGC), np.float32)
    for g in range(NGC):
        biasc[:, g] = bih[g * 128:(g + 1) * 128]
        if g < 8:
            biasc[:, g] += bhh[g * 128:(g + 1) * 128]
    bhhn = np.ascontiguousarray(bhh[2 * NH:].reshape(NJC, 128).T)
    i128 = np.eye(128, dtype=np.float32).astype(BF)

    # small-GRU packed weights (a-channel order = PAIRS via _chan perm)
    perm = np.array([_chan(u, k) for (u, k) in PAIRS])  # dev g -> canonical
    fam = (perm // NV)                                  # s-family per dev g
    gwih_s = [np.asarray(inp["ga_Wih"], np.float32),
              np.asarray(inp["go_Wih"], np.float32)]
    gwhh_s = [np.asarray(inp["ga_Whh"], np.float32),
              np.asarray(inp["go_Whh"], np.float32)]
    gbih_s = [np.asarray(inp["ga_bih"], np.float32),
              np.asarray(inp["go_bih"], np.float32)]
    gbhh_s = [np.asarray(inp["ga_bhh"], np.float32),
              np.asarray(inp["go_bhh"], np.float32)]
    gwih2 = np.zeros((NP, 3, NP), np.float32)
    gwhh2 = np.zeros((NP, 3, NP), np.float32)
    for p in range(NP):
        sp = p // NV
        i = p % NV
        for gate in range(3):
            row = gwih_s[sp][gate * NV + i]          # (NV,) canonical cols
            for gdev in range(NP):
                if fam[gdev] == sp:
                    gwih2[gdev, gate, p] = row[perm[gdev] % NV]
            gwhh2[sp * NV:(sp + 1) * NV, gate, p] = \
                gwhh_s[sp][gate * NV + i]
    gwih2 = gwih2.astype(BF)
    gwhh2 = gwhh2.astype(BF)
    biasr2 = np.zeros((NP, 1), np.float32)
    biasz2 = np.zeros((NP, 1), np.float32)
    biasn2 = np.zeros((NP, 1), np.float32)
    bhhn2 = np.zeros((NP, 1), np.float32)
    for s in range(2):
        sl = slice(s * NV, (s + 1) * NV)
        biasr2[sl, 0] = gbih_s[s][:NV] + gbhh_s[s][:NV]
        biasz2[sl, 0] = gbih_s[s][NV:2 * NV] + gbhh_s[s][NV:2 * NV]
        biasn2[sl, 0] = gbih_s[s][2 * NV:]
        bhhn2[sl, 0] = gbhh_s[s][2 * NV:]
    i80 = np.eye(NP, dtype=np.float32).astype(BF)

    vavo2 = np.zeros((NP, 2), np.float32)
    vavo2[:NV, 0] = np.asarray(inp["va"], np.float32)
    vavo2[NV:, 1] = np.asarray(inp["vo"], np.float32)
    vavo2 = vavo2.astype(BF)

    m0a = np.asarray(inp["m0_a"], np.float32)
    m0o = np.asarray(inp["m0_o"], np.float32)
    Us = [np.asarray(inp[n], np.float32) for n in ("Ua", "Va", "Uo", "Vo")]
    mvec = [m0a, m0o, m0o, m0a]
    um0p = np.zeros((128, NJC, NP), np.float32)
    for gdev, (u, k) in enumerate(PAIRS):
        Um = Us[u][k] @ mvec[u]                       # (NH,)
        for hc in range(NJC):
            um0p[:, hc, gdev] = Um[hc * 128:(hc + 1) * 128]
    um0p = um0p.astype(BF)

    Ma = np.asarray(inp["Ma"], np.float32)
    Mo = np.asarray(inp["Mo"], np.float32)
    tm = [np.tanh(m0a @ Ma).astype(np.float32),
          np.tanh(m0o @ Mo).astype(np.float32)]
    tmv32 = np.zeros((128, 32), np.float32)
    for s in range(2):
        for b in range(BS):
            for hc in range(NJC):
                tmv32[:, s * 16 + b * 4 + hc] = \
                    tm[s][hc * 128:(hc + 1) * 128]

    laloT2 = np.zeros((NP, 10), np.float32)
    laloT2[:NV, :5] = np.asarray(inp["la_W"], np.float32).T
    laloT2[NV:, 5:] = np.asarray(inp["lo_W"], np.float32).T
    laloT2 = laloT2.astype(BF)
    lalob2 = np.zeros((10, 1), np.float32)
    lalob2[:5, 0] = np.asarray(inp["la_b"], np.float32)
    lalob2[5:, 0] = np.asarray(inp["lo_b"], np.float32)
    onesg = np.zeros((10, 2), np.float32)
    onesg[:5, 0] = 1.0
    onesg[5:, 1] = 1.0
    onesg = onesg.astype(BF)

    shared = dict(wihT=wihT, whhT=whhT, biasc=biasc, bhhn=bhhn,
                  i128=i128, um0p=um0p, i80=i80, gwih2=gwih2,
                  gwhh2=gwhh2, biasr2=biasr2, biasz2=biasz2,
                  biasn2=biasn2, bhhn2=bhhn2, vavo2=vavo2,
                  tmv32=tmv32, laloT2=laloT2, lalob2=lalob2,
                  onesg=onesg)

    in_maps = []
    for c in range(NCORE):
        t0g = c * CHUNK - HALO_M
        xcm = np.zeros((BS, TM, KIN * 128), np.float32)
        lo = max(0, -t0g)
        hi = min(TM, T - t0g)
        xcm[:, lo:hi, :] = xpad[:, t0g + lo:t0g + hi, :]
        xT = np.ascontiguousarray(
            xcm.transpose(2, 0, 1).reshape(KIN, 128, BS, TM)).astype(BF)
        hmaskM = np.ones((128, BS, 128), np.float32)
        smaskSv = np.ones((NP, BS, 128), np.float32)
        if c == 0:
            hmaskM[:, :, :HALO_M] = 0.0
            smaskSv[:, :, :HALO_S] = 0.0
        tmlocv = np.zeros((2, BS, CHUNK), np.float32)
        w0, w1 = c * CHUNK, (c + 1) * CHUNK
        n_valid = max(0, min(seq, w1) - w0)
        tmlocv[:, :, :n_valid] = 1.0
        ustat = np.zeros((PSH, NJC, 128, NJC, 128), np.float32)
        for i in range(PSH):
            u, k = PAIRS[c * PSH + i]
            # ustat[i, jc, q, hc, p] = U_u[k, hc*128+p, jc*128+q]
            ustat[i] = Us[u][k].reshape(NJC, 128, NJC, 128).transpose(
                2, 3, 0, 1)
        m = dict(shared)
        m.update(xT=xT, hmaskM=hmaskM, smaskS=smaskSv, tmloc=tmlocv,
                 ustat=ustat.astype(BF))
        in_maps.append(m)
    return in_maps


def kernel(**inputs):
    debug = bool(int(os.environ.get("CMLA_DEBUG", "0")))
    key = ("prog", debug, ITERS_MAIN, ITERS_SMALL)
    if key not in _CACHE:
        _CACHE[key] = build_program(debug=debug)
    nc = _CACHE[key]
    in_maps = _prep_inputs(inputs)
    res = run_bass_kernel_spmd(
        nc, in_maps, list(range(NCORE)),
        trace=bool(int(os.environ.get("CMLA_TRACE", "0"))))
    _CACHE["last_results"] = res
    ya = np.zeros((BS, T, NC), np.float32)
    yo = np.zeros((BS, T, NC), np.float32)
    for c in range(NCORE):
        y = res.results[c]["y"]
        ya[:, c * CHUNK:(c + 1) * CHUNK, :] = y[:5].transpose(1, 2, 0)
        yo[:, c * CHUNK:(c + 1) * CHUNK, :] = y[5:].transpose(1, 2, 0)
    return ya, yo


# revision 21
# speedup vs baseline: 1.2006x; 1.2006x over previous
"""CMLANet Trainium2 kernel: 8-core SPMD, time-sharded, bf16 DEER.

Main GRU and the two small attention GRUs run scan-accelerated
fixed-point (DEER) iterations: gates from the previous iterate via bf16
PE matmuls, then the linear recurrence h_t = z_t h_{t-1} + (1-z_t) n_t
applied exactly with the fp32 hardware prefix scan.  Time is sharded
over 8 cores (256 steps each) with short halos; core 0's halo is pinned
to the true zero initial state by masking the scan inputs.  All four
batch samples are scanned in ONE scan instruction per state group: a
zero reset slot between batch blocks re-zeroes the carried state.  The
two small GRUs (a/o) are packed into 80 partitions with
block-structured weights.  The bilinear U@m1 products are
topic-sharded across cores (10 of 80 (u,k) pairs each, msel-uniform
slots) and all-gathered; softmax normalization is folded into one
AllReduce via the unnormalized-exp trick.
"""

import os
import sys
import numpy as np
import ml_dtypes

sys.path.insert(0, "/opt/trn_rl_repo")

import concourse.bass as bass  # noqa: E402,F401
import concourse.bacc as bacc  # noqa: E402
import concourse.tile as tile  # noqa: E402
from concourse import mybir  # noqa: E402
from concourse.bass_utils import run_bass_kernel_spmd  # noqa: E402

F32 = mybir.dt.float32
BF16 = mybir.dt.bfloat16
ALU = mybir.AluOpType
AF = mybir.ActivationFunctionType
AX = mybir.AxisListType

BS, T, CS, DE, NH, NT, NC = 4, 2048, 5, 300, 512, 20, 5
NV = 2 * NT          # 40
NP = 2 * NV          # 80 packed (s=0, s=1)
NIN = DE * CS        # 1500
KIN = 12
G = 3 * NH
NGC = 12
NJC = 4
NCORE = 8
CHUNK = T // NCORE   # 256
HALO_M = 48
TM = CHUNK + HALO_M  # 304
HALO_S = 32
TS = CHUNK + HALO_S  # 288
ITERS_MAIN = int(os.environ.get("CMLA_ITERS_MAIN", "5"))
ITERS_SMALL = int(os.environ.get("CMLA_ITERS_SMALL", "4"))
PSH = 10             # (u,k) pairs per core
FM = BS * (TM + 1)   # flat scan span, main
FS = BS * (TS + 1)   # flat scan span, small

TCS = [(0, 128), (128, 128), (256, 48)]
STCS = [(0, 128), (128, 128), (256, 32)]

_CACHE = {}


def build_program(debug=False):
    nc = bacc.Bacc("TRN2", target_bir_lowering=False, debug=False,
                   num_devices=NCORE)

    def din(name, shape, dt=F32):
        return nc.dram_tensor(name, list(shape), dt,
                              kind="ExternalInput").ap()

    d = {}
    d["xT"] = din("xT", [KIN, 128, BS, TM], BF16)
    d["wihT"] = din("wihT", [NGC, 128, KIN, 128], BF16)
    d["whhT"] = din("whhT", [NJC, 128, NGC, 128], BF16)
    d["biasc"] = din("biasc", [128, NGC])
    d["bhhn"] = din("bhhn", [128, NJC])
    d["hmaskM"] = din("hmaskM", [128, BS, 128])
    d["i128"] = din("i128", [128, 128], BF16)
    d["um0p"] = din("um0p", [128, NJC, NP], BF16)
    d["i80"] = din("i80", [NP, NP], BF16)
    d["gwih2"] = din("gwih2", [NP, 3, NP], BF16)
    d["gwhh2"] = din("gwhh2", [NP, 3, NP], BF16)
    d["biasr2"] = din("biasr2", [NP, 1])
    d["biasz2"] = din("biasz2", [NP, 1])
    d["biasn2"] = din("biasn2", [NP, 1])
    d["bhhn2"] = din("bhhn2", [NP, 1])
    d["smaskS"] = din("smaskS", [NP, BS, 128])
    d["vavo2"] = din("vavo2", [NP, 2], BF16)
    d["tmloc"] = din("tmloc", [2, BS, CHUNK])
    d["tmv32"] = din("tmv32", [128, 32])
    d["ustat"] = din("ustat", [PSH, NJC, 128, NJC, 128], BF16)
    d["laloT2"] = din("laloT2", [NP, 10], BF16)
    d["lalob2"] = din("lalob2", [10, 1])
    d["onesg"] = din("onesg", [10, 2], BF16)

    d["y"] = nc.dram_tensor("y", [10, BS, CHUNK], F32,
                            kind="ExternalOutput").ap()
    if debug:
        d["hdbg"] = nc.dram_tensor("hdbg", [NJC, 128, BS, TM], F32,
                                   kind="ExternalOutput").ap()
        d["r1dbg"] = nc.dram_tensor("r1dbg", [NP, BS, TS], F32,
                                    kind="ExternalOutput").ap()
        d["m1dbg"] = nc.dram_tensor("m1dbg", [128, 32], F32,
                                    kind="ExternalOutput").ap()

    d["cc2_in"] = nc.dram_tensor("cc2_in", [128, 36], F32)
    d["cc2_out"] = nc.dram_tensor("cc2_out", [128, 36], F32,
                                  addr_space="Shared")
    d["cc3_in"] = nc.dram_tensor("cc3_in", [128, PSH * 16], F32)
    d["cc3_out"] = nc.dram_tensor("cc3_out", [NCORE, 128, PSH * 16], F32,
                                  addr_space="Shared")

    with tile.TileContext(nc, num_cores=NCORE) as tc:
        _body(nc, tc, d, debug)
    nc.compile()
    return nc


def _body(nc, tc, d, debug):
    from contextlib import ExitStack
    es = ExitStack()
    ex = es.enter_context(tc.tile_pool(name="persist", bufs=1))

    # persistent state, flat layout: col 0 is the zero init; batch b's
    # h_t lives at flat col 1 + b*(TM+1) + t; the scan's reset slot
    # (t = TM) writes the 0 that serves as batch b+1's h_{-1}.
    Zf = [ex.tile([128, FM + 1], F32, name=f"Zf{j}") for j in range(NJC)]
    Zb = [ex.tile([128, FM + 1], BF16, name=f"Zb{j}") for j in range(NJC)]
    for j in range(NJC):
        nc.vector.memset(Zf[j][:, :], 0.0)
        nc.vector.memset(Zb[j][:, :], 0.0)

    def prevv(t):          # h_{t-1} view: [128, BS, TM+1]
        return t[:, 0:FM].rearrange("p (b t) -> p b t", b=BS)

    def valv(t):           # h_t view: [128, BS, TM+1]
        return t[:, 1:FM + 1].rearrange("p (b t) -> p b t", b=BS)

    # phase-3 constants, loaded up front so later phases never wait
    um0p = ex.tile([128, NJC, NP], BF16, name="um0pt")
    i80 = ex.tile([NP, NP], BF16, name="i80t")
    gwih2 = ex.tile([NP, 3, NP], BF16, name="gwih2t")
    gwhh2 = ex.tile([NP, 3, NP], BF16, name="gwhh2t")
    biasr2 = ex.tile([NP, 1], F32, name="biasr2t")
    biasz2 = ex.tile([NP, 1], F32, name="biasz2t")
    biasn2 = ex.tile([NP, 1], F32, name="biasn2t")
    bhhn2 = ex.tile([NP, 1], F32, name="bhhn2t")
    smaskS = ex.tile([NP, BS, 128], F32, name="smaskSt")
    vavo2 = ex.tile([NP, 2], BF16, name="vavo2t")
    tmloc = ex.tile([2, BS, CHUNK], F32, name="tmloct")
    tmv32 = ex.tile([128, 32], F32, name="tmv32t")
    for name, t in (("um0p", um0p), ("i80", i80), ("gwih2", gwih2),
                    ("gwhh2", gwhh2), ("biasr2", biasr2),
                    ("biasz2", biasz2), ("biasn2", biasn2),
                    ("bhhn2", bhhn2), ("smaskS", smaskS),
                    ("vavo2", vavo2), ("tmloc", tmloc),
                    ("tmv32", tmv32)):
        if len(t.shape) == 2:
            nc.sync.dma_start(t[:, :], d[name])
        else:
            nc.sync.dma_start(t[:, :, :], d[name])

    # ======================= main GRU =======================
    with tc.tile_pool(name="xwpool", bufs=1) as xp:
        xwr = [xp.tile([128, BS, TM], BF16, name=f"xwr{g}")
               for g in range(8)]          # r: 0-3, z: 4-7 (bias folded)
        xwn = [xp.tile([128, BS, TM], F32, name=f"xwn{j}")
               for j in range(NJC)]
        i128 = xp.tile([128, 128], BF16, name="i128t")
        biasc = xp.tile([128, NGC], F32, name="biasct")
        bhhn = xp.tile([128, NJC], F32, name="bhhnt")
        hmaskM = xp.tile([128, BS, 128], F32, name="hmaskMt")
        nc.sync.dma_start(i128[:, :], d["i128"])
        nc.sync.dma_start(biasc[:, :], d["biasc"])
        nc.sync.dma_start(bhhn[:, :], d["bhhn"])
        nc.sync.dma_start(hmaskM[:, :, :], d["hmaskM"])

        # ---- xw = x @ Wih^T (+bih, +bhh for r,z) ----
        with tc.tile_pool(name="xstream", bufs=2) as st, \
                tc.tile_pool(name="wihpool", bufs=1) as wp, \
                tc.tile_pool(name="ps_xw", bufs=3, space="PSUM") as ps:
            wih = wp.tile([128, NGC, KIN, 128], BF16, name="wih")
            for g in range(NGC):
                nc.sync.dma_start(wih[:, g, :, :], d["wihT"][g])
            for (t0, tw) in TCS:
                xs = st.tile([128, KIN, BS, tw], BF16, name="xs", tag="xs")
                for kc in range(KIN):
                    nc.sync.dma_start(xs[:, kc, :, :],
                                      d["xT"][kc, :, :, t0:t0 + tw])
                for g in range(NGC):
                    p = ps.tile([128, BS, tw], F32, name="xwp", tag="xwp")
                    for kc in range(KIN):
                        nc.tensor.matmul(p[:, :, :], wih[:, g, kc, :],
                                         xs[:, kc, :, :],
                                         start=(kc == 0),
                                         stop=(kc == KIN - 1))
                    o = xwr[g] if g < 8 else xwn[g - 8]
                    nc.scalar.activation(o[:, :, t0:t0 + tw], p[:, :, :],
                                         AF.Identity,
                                         bias=biasc[:, g:g + 1])

        # ---- DEER iterations ----
        with tc.tile_pool(name="sc_deer", bufs=2) as sc, \
                tc.tile_pool(name="zbpool", bufs=1) as zb, \
                tc.tile_pool(name="ps_deer", bufs=2, space="PSUM") as ps:
            whh = sc.tile([128, NJC, NGC, 128], BF16, name="whh")
            nc.sync.dma_start(whh[:, :, :, :],
                              d["whhT"].rearrange("j q g p -> q j g p"))
            zcw = [zb.tile([128, BS, TM + 1], F32, name=f"zcw{j}")
                   for j in range(NJC)]
            bcw = [zb.tile([128, BS, TM + 1], F32, name=f"bcw{j}")
                   for j in range(NJC)]
            for j in range(NJC):
                nc.vector.memset(zcw[j][:, :, :], 0.0)
                nc.vector.memset(bcw[j][:, :, :], 0.0)

            def gate_tail(j, t0, tw, r_src, z_src, n_src):
                # r/z activations, n-path, bm into bcw (masked on chunk 0)
                r = sc.tile([128, BS, tw], F32, name="r", tag="r")
                nc.scalar.activation(r[:, :, :], r_src, AF.Sigmoid)
                nc.scalar.activation(zcw[j][:, :, t0:t0 + tw], z_src,
                                     AF.Sigmoid)
                npre = sc.tile([128, BS, tw], F32, name="npre", tag="npre")
                if n_src is None:
                    nc.vector.scalar_tensor_tensor(
                        npre[:, :, :], r[:, :, :], bhhn[:, j:j + 1],
                        xwn[j][:, :, t0:t0 + tw], ALU.mult, ALU.add)
                else:
                    rhn = sc.tile([128, BS, tw], F32, name="rhn",
                                  tag="rhn")
                    nc.vector.scalar_tensor_tensor(
                        rhn[:, :, :], n_src, bhhn[:, j:j + 1], r[:, :, :],
                        ALU.add, ALU.mult)
                    nc.vector.tensor_add(npre[:, :, :], rhn[:, :, :],
                                         xwn[j][:, :, t0:t0 + tw])
                nt_ = sc.tile([128, BS, tw], F32, name="nt", tag="nt")
                nc.scalar.activation(nt_[:, :, :], npre[:, :, :], AF.Tanh)
                if t0 == 0:
                    bm = sc.tile([128, BS, tw], F32, name="bm", tag="bm")
                    nc.vector.scalar_tensor_tensor(
                        bm[:, :, :], zcw[j][:, :, t0:t0 + tw], 1.0,
                        nt_[:, :, :], ALU.subtract, ALU.mult)
                    nc.vector.tensor_mul(bcw[j][:, :, t0:t0 + tw],
                                         bm[:, :, :], hmaskM[:, :, :])
                else:
                    nc.vector.scalar_tensor_tensor(
                        bcw[j][:, :, t0:t0 + tw],
                        zcw[j][:, :, t0:t0 + tw], 1.0, nt_[:, :, :],
                        ALU.subtract, ALU.mult)

            def scan_and_cast():
                for j in range(NJC):
                    nc.vector.tensor_tensor_scan(
                        Zf[j][:, 1:FM + 1],
                        zcw[j][:, :, :].rearrange("p b t -> p (b t)"),
                        bcw[j][:, :, :].rearrange("p b t -> p (b t)"),
                        Zf[j][:, 0:1], ALU.mult, ALU.subtract)
                for j in range(NJC):
                    nc.scalar.copy(Zb[j][:, :], Zf[j][:, :])

            # iteration 0: H = 0, no matmuls
            for (t0, tw) in TCS:
                for j in range(NJC):
                    gate_tail(j, t0, tw, xwr[j][:, :, t0:t0 + tw],
                              xwr[4 + j][:, :, t0:t0 + tw], None)
            scan_and_cast()

            for it in range(1, ITERS_MAIN):
                for (t0, tw) in TCS:
                    for j in range(NJC):
                        pr = ps.tile([128, BS, tw], F32, name="pr",
                                     tag="pr")
                        pz = ps.tile([128, BS, tw], F32, name="pz",
                                     tag="pz")
                        pn = ps.tile([128, BS, tw], F32, name="pn",
                                     tag="pn")
                        nc.tensor.matmul(pr[:, :, :], i128[:, :],
                                         xwr[j][:, :, t0:t0 + tw],
                                         start=True, stop=False)
                        nc.tensor.matmul(pz[:, :, :], i128[:, :],
                                         xwr[4 + j][:, :, t0:t0 + tw],
                                         start=True, stop=False)
                        for jc in range(NJC):
                            hs = prevv(Zb[jc])[:, :, t0:t0 + tw]
                            nc.tensor.matmul(pr[:, :, :],
                                             whh[:, jc, j, :], hs,
                                             start=False,
                                             stop=(jc == NJC - 1))
                            nc.tensor.matmul(pz[:, :, :],
                                             whh[:, jc, 4 + j, :], hs,
                                             start=False,
                                             stop=(jc == NJC - 1))
                            nc.tensor.matmul(pn[:, :, :],
                                             whh[:, jc, 8 + j, :], hs,
                                             start=(jc == 0),
                                             stop=(jc == NJC - 1))
                        gate_tail(j, t0, tw, pr[:, :, :], pz[:, :, :],
                                  pn[:, :, :])
                scan_and_cast()

        if debug:
            for j in range(NJC):
                nc.sync.dma_start(d["hdbg"][j], valv(Zf[j])[:, :, 0:TM])

    # ================== attention + small GRUs ==================
    ex2 = es.enter_context(tc.tile_pool(name="persist2", bufs=1))
    us = ex2.tile([128, PSH, NJC, NJC, 128], BF16, name="us")
    nc.sync.dma_start(us[:, :, :, :, :],
                      d["ustat"].rearrange("i jc q hc p -> q i jc hc p"))

    def spv(t):            # small h_{t-1} view [NP, BS, TS+1]
        return t[:, 0:FS].rearrange("p (b t) -> p b t", b=BS)

    def svv(t):            # small h_t view [NP, BS, TS+1]
        return t[:, 1:FS + 1].rearrange("p (b t) -> p b t", b=BS)

    def tp_gates(lhs_of, sxw_r, sxw_z, sxw_n, perb):
        """a = tanh(sum_hc lhsT_hc @ h); small-gate xw tiles."""
        with tc.tile_pool(name="sc_tp", bufs=2) as sc, \
                tc.tile_pool(name="ps_tp", bufs=2, space="PSUM") as ps:
            for (q0, qw) in STCS:
                a = sc.tile([NP, BS, qw], BF16, name="a", tag="a")
                if perb:
                    for b in range(BS):
                        pa = ps.tile([NP, qw], F32, name="pa", tag="pa")
                        for hc in range(NJC):
                            nc.tensor.matmul(
                                pa[:, :], lhs_of(hc, b),
                                valv(Zb[hc])[:, b, 16 + q0:16 + q0 + qw],
                                start=(hc == 0), stop=(hc == NJC - 1))
                        nc.scalar.activation(a[:, b, :], pa[:, :],
                                             AF.Tanh)
                else:
                    pa = ps.tile([NP, BS, qw], F32, name="pab",
                                 tag="pab")
                    for hc in range(NJC):
                        nc.tensor.matmul(
                            pa[:, :, :], lhs_of(hc, None),
                            valv(Zb[hc])[:, :, 16 + q0:16 + q0 + qw],
                            start=(hc == 0), stop=(hc == NJC - 1))
                    nc.scalar.activation(a[:, :, :], pa[:, :, :],
                                         AF.Tanh)
                for gi, (xwg, bsl) in enumerate(
                        ((sxw_r, biasr2), (sxw_z, biasz2),
                         (sxw_n, biasn2))):
                    px = ps.tile([NP, BS, qw], F32, name="px", tag="px")
                    nc.tensor.matmul(px[:, :, :], gwih2[:, gi, :],
                                     a[:, :, :], start=True, stop=True)
                    nc.scalar.activation(xwg[:, :, q0:q0 + qw],
                                         px[:, :, :], AF.Identity,
                                         bias=bsl[:, 0:1])

    def small_deer(sxw_r, sxw_z, sxw_n, Rf, Rb):
        with tc.tile_pool(name="sc_sd", bufs=2) as sc, \
                tc.tile_pool(name="zbs", bufs=1) as zbs, \
                tc.tile_pool(name="ps_sd", bufs=2, space="PSUM") as ps:
            zcs = zbs.tile([NP, BS, TS + 1], F32, name="zcs")
            bcs = zbs.tile([NP, BS, TS + 1], F32, name="bcs")
            rw = zbs.tile([NP, BS, TS], F32, name="rw")
            rhnw = zbs.tile([NP, BS, TS], F32, name="rhnw")
            nc.vector.memset(zcs[:, :, :], 0.0)
            nc.vector.memset(bcs[:, :, :], 0.0)

            def window_tail(first):
                # n-path + bm, batched over the whole window
                npre = sc.tile([NP, BS, TS], F32, name="snpre",
                               tag="snpre")
                if first:
                    nc.vector.scalar_tensor_tensor(
                        npre[:, :, :], rw[:, :, :], bhhn2[:, 0:1],
                        sxw_n[:, :, :], ALU.mult, ALU.add)
                else:
                    nc.vector.tensor_add(npre[:, :, :], rhnw[:, :, :],
                                         sxw_n[:, :, :])
                nt_ = sc.tile([NP, BS, TS], F32, name="snt", tag="snt")
                nc.scalar.activation(nt_[:, :, :], npre[:, :, :],
                                     AF.Tanh)
                bm = sc.tile([NP, BS, TS], F32, name="sbm", tag="sbm")
                nc.vector.scalar_tensor_tensor(
                    bm[:, :, :], zcs[:, :, 0:TS], 1.0, nt_[:, :, :],
                    ALU.subtract, ALU.mult)
                nc.vector.tensor_mul(bm[:, :, 0:128], bm[:, :, 0:128],
                                     smaskS[:, :, :])
                nc.vector.tensor_copy(bcs[:, :, 0:TS], bm[:, :, :])

            def scan_cast():
                nc.vector.tensor_tensor_scan(
                    Rf[:, 1:FS + 1],
                    zcs[:, :, :].rearrange("p b t -> p (b t)"),
                    bcs[:, :, :].rearrange("p b t -> p (b t)"),
                    Rf[:, 0:1], ALU.mult, ALU.subtract)
                nc.scalar.copy(Rb[:, :], Rf[:, :])

            # iteration 0 (R = 0): whole-window activations
            nc.scalar.activation(rw[:, :, :], sxw_r[:, :, :], AF.Sigmoid)
            nc.scalar.activation(zcs[:, :, 0:TS], sxw_z[:, :, :],
                                 AF.Sigmoid)
            window_tail(True)
            scan_cast()
            for it in range(1, ITERS_SMALL):
                for (q0, qw) in STCS:
                    p_r = ps.tile([NP, BS, qw], F32, name="p_r",
                                  tag="p_r")
                    p_z = ps.tile([NP, BS, qw], F32, name="p_z",
                                  tag="p_z")
                    p_n = ps.tile([NP, BS, qw], F32, name="p_n",
                                  tag="p_n")
                    rs = spv(Rb)[:, :, q0:q0 + qw]
                    nc.tensor.matmul(p_r[:, :, :], i80[:, :],
                                     sxw_r[:, :, q0:q0 + qw],
                                     start=True, stop=False)
                    nc.tensor.matmul(p_r[:, :, :], gwhh2[:, 0, :], rs,
                                     start=False, stop=True)
                    nc.tensor.matmul(p_z[:, :, :], i80[:, :],
                                     sxw_z[:, :, q0:q0 + qw],
                                     start=True, stop=False)
                    nc.tensor.matmul(p_z[:, :, :], gwhh2[:, 1, :], rs,
                                     start=False, stop=True)
                    nc.tensor.matmul(p_n[:, :, :], gwhh2[:, 2, :], rs,
                                     start=True, stop=True)
                    r_ = sc.tile([NP, BS, qw], F32, name="sr", tag="sr")
                    nc.scalar.activation(r_[:, :, :], p_r[:, :, :],
                                         AF.Sigmoid)
                    nc.scalar.activation(zcs[:, :, q0:q0 + qw],
                                         p_z[:, :, :], AF.Sigmoid)
                    nc.vector.scalar_tensor_tensor(
                        rhnw[:, :, q0:q0 + qw], p_n[:, :, :],
                        bhhn2[:, 0:1], r_[:, :, :], ALU.add, ALU.mult)
                window_tail(False)
                scan_cast()

    # ---- phase 1 ----
    sxw_r1 = ex2.tile([NP, BS, TS], BF16, name="sxwr1")
    sxw_z1 = ex2.tile([NP, BS, TS], BF16, name="sxwz1")
    sxw_n1 = ex2.tile([NP, BS, TS], F32, name="sxwn1")
    tp_gates(lambda hc, b: um0p[:, hc, :], sxw_r1, sxw_z1, sxw_n1, False)
    Rf1 = ex2.tile([NP, FS + 1], F32, name="Rf1")
    Rb1 = ex2.tile([NP, FS + 1], BF16, name="Rb1")
    nc.vector.memset(Rf1[:, :], 0.0)
    nc.vector.memset(Rb1[:, :], 0.0)
    small_deer(sxw_r1, sxw_z1, sxw_n1, Rf1, Rb1)
    if debug:
        nc.sync.dma_start(d["r1dbg"], svv(Rf1)[:, :, 0:TS])

    # ---- attention middle: unnormalized-exp softmax + context ----
    m1t = ex2.tile([128, 2, BS, NJC], F32, name="m1t")
    with tc.tile_pool(name="sc_att", bufs=1) as sc, \
            tc.tile_pool(name="ps_att", bufs=2, space="PSUM") as ps:
        w = sc.tile([2, BS, CHUNK], F32, name="w")
        for (o0, ow) in ((0, 128), (128, 128)):
            pv = ps.tile([2, BS, ow], F32, name="pv", tag="pv")
            nc.tensor.matmul(
                pv[:, :, :], vavo2[:, :],
                svv(Rb1)[:, :, HALO_S + o0:HALO_S + o0 + ow],
                start=True, stop=True)
            nc.scalar.activation(w[:, :, o0:o0 + ow], pv[:, :, :],
                                 AF.Exp)
        nc.vector.tensor_mul(w[:, :, :], w[:, :, :], tmloc[:, :, :])
        cc2t = sc.tile([128, 36], F32, name="cc2t")
        nc.vector.memset(cc2t[:, :], 0.0)
        for b in range(BS):
            nc.vector.tensor_reduce(cc2t[0:2, 32 + b:32 + b + 1],
                                    w[:, b, :], AX.X, ALU.add)
        for s in range(2):
            for b in range(BS):
                row = sc.tile([1, CHUNK], F32, name="row", tag="row")
                nc.sync.dma_start(row[:, :], w[s:s + 1, b, :])
                wb = sc.tile([128, CHUNK], F32, name="wb", tag="wb")
                nc.gpsimd.partition_broadcast(wb[:, :], row[0:1, :])
                for hc in range(NJC):
                    prod = sc.tile([128, CHUNK], F32, name="prod",
                                   tag="prod")
                    nc.vector.tensor_mul(
                        prod[:, :],
                        valv(Zf[hc])[:, b, HALO_M:HALO_M + CHUNK],
                        wb[:, :])
                    col = s * 16 + b * 4 + hc
                    nc.vector.tensor_reduce(cc2t[:, col:col + 1],
                                            prod[:, :], AX.X, ALU.add)
        nc.sync.dma_start(d["cc2_in"].ap(), cc2t[:, :])
        nc.gpsimd.collective_compute(
            "AllReduce", ALU.add, replica_groups=[list(range(NCORE))],
            ins=[d["cc2_in"].ap()], outs=[d["cc2_out"].ap()])
        car = sc.tile([128, 36], F32, name="car")
        nc.sync.dma_start(car[:, :], d["cc2_out"].ap())
        rsw = sc.tile([2, BS], F32, name="rsw")
        nc.vector.reciprocal(rsw[:, :], car[0:2, 32:36])
        for s in range(2):
            for b in range(BS):
                rb1 = sc.tile([1, 1], F32, name="rb1", tag="rb1")
                nc.sync.dma_start(rb1[:, :], rsw[s:s + 1, b:b + 1])
                rbb = sc.tile([128, 1], F32, name="rbb", tag="rbb")
                nc.gpsimd.partition_broadcast(rbb[:, :], rb1[0:1, :])
                col = s * 16 + b * 4
                sc_car = sc.tile([128, NJC], F32, name="sc_car",
                                 tag="sc_car")
                nc.vector.tensor_scalar_mul(sc_car[:, :],
                                            car[:, col:col + NJC],
                                            rbb[:, 0:1])
                nc.vector.tensor_add(m1t[:, s, b, :], sc_car[:, :],
                                     tmv32[:, col:col + NJC])
        if debug:
            nc.sync.dma_start(
                d["m1dbg"],
                m1t[:, :, :, :].rearrange("p s b h -> p (s b h)"))

    # ---- um1 = U @ m1, topic-sharded; all-gather ----
    um1g = ex2.tile([128, NCORE, PSH, NJC, BS], BF16, name="um1g")
    m1b = ex2.tile([128, 2, BS, NJC], BF16, name="m1b")
    nc.scalar.copy(m1b[:, :, :, :], m1t[:, :, :, :])
    with tc.tile_pool(name="sc_um", bufs=2) as sc, \
            tc.tile_pool(name="ps_um", bufs=2, space="PSUM") as ps:
        cc3t = sc.tile([128, PSH, NJC, BS], F32, name="cc3t")
        for i in range(PSH):
            m = 0 if i < 5 else 1
            for hc in range(NJC):
                pu = ps.tile([128, BS], F32, name="pu", tag="pu")
                for jc in range(NJC):
                    nc.tensor.matmul(pu[:, :], us[:, i, jc, hc, :],
                                     m1b[:, m, :, jc],
                                     start=(jc == 0),
                                     stop=(jc == NJC - 1))
                nc.scalar.copy(cc3t[:, i, hc, :], pu[:, :])
        nc.sync.dma_start(
            d["cc3_in"].ap(),
            cc3t[:, :, :, :].rearrange("p i h b -> p (i h b)"))
        nc.gpsimd.collective_compute(
            "AllGather", ALU.bypass, replica_groups=[list(range(NCORE))],
            ins=[d["cc3_in"].ap()], outs=[d["cc3_out"].ap()])
        umf = sc.tile([128, NCORE, PSH, NJC, BS], F32, name="umf")
        nc.sync.dma_start(
            umf[:, :, :, :, :],
            d["cc3_out"].ap().rearrange("c q x -> q c x"))
        nc.scalar.copy(um1g[:, :, :, :, :], umf[:, :, :, :, :])

    # ---- phase 2 ----
    sxw_r2 = ex2.tile([NP, BS, TS], BF16, name="sxwr2")
    sxw_z2 = ex2.tile([NP, BS, TS], BF16, name="sxwz2")
    sxw_n2 = ex2.tile([NP, BS, TS], F32, name="sxwn2")
    tp_gates(lambda hc, b: um1g[:, :, :, hc, b], sxw_r2, sxw_z2,
             sxw_n2, True)
    Rf2 = ex2.tile([NP, FS + 1], F32, name="Rf2")
    Rb2 = ex2.tile([NP, FS + 1], BF16, name="Rb2")
    nc.vector.memset(Rf2[:, :], 0.0)
    nc.vector.memset(Rb2[:, :], 0.0)
    small_deer(sxw_r2, sxw_z2, sxw_n2, Rf2, Rb2)

    # ---- final: ha = R1+R2; logits; per-class softmax ----
    with tc.tile_pool(name="sc_fin", bufs=2) as sc, \
            tc.tile_pool(name="ps_fin", bufs=2, space="PSUM") as ps:
        lal = sc.tile([NP, 10], BF16, name="lal")
        lb = [sc.tile([5, 1], F32, name=f"lb{s}") for s in range(2)]
        ones5 = sc.tile([5, 1], BF16, name="ones5t")
        nc.sync.dma_start(lal[:, :], d["laloT2"])
        for s in range(2):
            nc.sync.dma_start(lb[s][:, :], d["lalob2"][5 * s:5 * s + 5])
        nc.sync.dma_start(ones5[:, :], d["onesg"][0:5, 0:1])
        ha = sc.tile([NP, BS, CHUNK], BF16, name="ha")
        nc.vector.tensor_add(ha[:, :, :],
                             svv(Rf1)[:, :, HALO_S:HALO_S + CHUNK],
                             svv(Rf2)[:, :, HALO_S:HALO_S + CHUNK])
        yt = [sc.tile([5, BS, CHUNK], F32, name=f"yt{s}")
              for s in range(2)]
        for (o0, ow) in ((0, 128), (128, 128)):
            for s in range(2):
                pl = ps.tile([5, BS, ow], F32, name="pl", tag="pl")
                nc.tensor.matmul(pl[:, :, :], lal[:, 5 * s:5 * s + 5],
                                 ha[:, :, o0:o0 + ow], start=True,
                                 stop=True)
                el = sc.tile([5, BS, ow], F32, name="el", tag="el")
                nc.scalar.activation(el[:, :, :], pl[:, :, :], AF.Exp,
                                     bias=lb[s][:, 0:1])
                elb = sc.tile([5, BS, ow], BF16, name="elb", tag="elb")
                nc.vector.tensor_copy(elb[:, :, :], el[:, :, :])
                pss = ps.tile([1, BS, ow], F32, name="pss", tag="pss")
                nc.tensor.matmul(pss[:, :, :], ones5[:, :],
                                 elb[:, :, :], start=True, stop=True)
                rs = sc.tile([1, BS, ow], F32, name="rs", tag="rs")
                nc.vector.reciprocal(rs[:, :, :], pss[:, :, :])
                rb = sc.tile([5, BS, ow], F32, name="rb", tag="rb")
                nc.gpsimd.partition_broadcast(rb[:, :, :], rs[:, :, :],
                                              channels=5)
                nc.vector.tensor_mul(yt[s][:, :, o0:o0 + ow],
                                     el[:, :, :], rb[:, :, :])
        for s in range(2):
            nc.sync.dma_start(d["y"][5 * s:5 * s + 5], yt[s][:, :, :])
    es.close()


# ----------------------------------------------------------------------------
# host side
# ----------------------------------------------------------------------------

BF = ml_dtypes.bfloat16

# slot -> (u, k) pair assignment: slots 0-4 use m=0 pairs, 5-9 use m=1,
# uniform across cores (required for SPMD).  u: 0=Ua 1=Va 2=Uo 3=Vo.
_M0 = [(0, k) for k in range(NT)] + [(3, k) for k in range(NT)]
_M1 = [(1, k) for k in range(NT)] + [(2, k) for k in range(NT)]
PAIRS = []
for _c in range(NCORE):
    PAIRS += _M0[_c * 5:(_c + 1) * 5] + _M1[_c * 5:(_c + 1) * 5]


def _chan(u, k):
    """Canonical packed-80 a-channel for pair (u, k)."""
    s = 0 if u < 2 else 1
    within = k if u in (0, 2) else NT + k
    return s * NV + within


def _prep_inputs(inputs):
    inp = {k: (np.asarray(v) if not np.isscalar(v) else v)
           for k, v in inputs.items()}
    emb = np.asarray(inp["emb"], np.float32)
    idx = np.asarray(inp["index_embed"])
    cw = np.asarray(inp["context_words"])
    seq = int(np.asarray(inp["seq_size"]))

    tok = emb.T[idx]
    pad = np.broadcast_to(np.asarray(inp["padding"], np.float32),
                          (BS, 1, DE))
    pkt = np.broadcast_to(np.asarray(inp["punkt"], np.float32),
                          (BS, 1, DE))
    nodes = np.concatenate([tok, pad, pkt], axis=1).astype(np.float32)
    x = np.stack([nodes[b][cw[b]] for b in range(BS)]).reshape(BS, T, NIN)
    xpad = np.zeros((BS, T, KIN * 128), np.float32)
    xpad[:, :, :NIN] = x

    Wih = np.asarray(inp["gru_Wih"], np.float32)
    Whh = np.asarray(inp["gru_Whh"], np.float32)
    bih = np.asarray(inp["gru_bih"], np.float32)
    bhh = np.asarray(inp["gru_bhh"], np.float32)

    wpd = np.zeros((G, KIN * 128), np.float32)
    wpd[:, :NIN] = Wih
    # wihT[g, q, k, p] = Wih[g*128+p, k*128+q]
    wihT = np.ascontiguousarray(
        wpd.reshape(NGC, 128, KIN, 128).transpose(0, 3, 2, 1)).astype(BF)
    whhT = np.ascontiguousarray(
        Whh.reshape(NGC, 128, NJC, 128).transpose(2, 3, 0, 1)).astype(BF)
    biasc = np.zeros((128, NGC), np.float32)
    for g in range(NGC):
        biasc[:, g] = bih[g * 128:(g + 1) * 128]
        if g < 8:
            biasc[:, g] += bhh[g * 128:(g + 1) * 128]
    bhhn = np.ascontiguousarray(bhh[2 * NH:].reshape(NJC, 128).T)
    i128 = np.eye(128, dtype=np.float32).astype(BF)

    # small-GRU packed weights (a-channel order = PAIRS via _chan perm)
    perm = np.array([_chan(u, k) for (u, k) in PAIRS])
    fam = (perm // NV)
    gwih_s = [np.asarray(inp["ga_Wih"], np.float32),
              np.asarray(inp["go_Wih"], np.float32)]
    gwhh_s = [np.asarray(inp["ga_Whh"], np.float32),
              np.asarray(inp["go_Whh"], np.float32)]
    gbih_s = [np.asarray(inp["ga_bih"], np.float32),
              np.asarray(inp["go_bih"], np.float32)]
    gbhh_s = [np.asarray(inp["ga_bhh"], np.float32),
              np.asarray(inp["go_bhh"], np.float32)]
    gwih2 = np.zeros((NP, 3, NP), np.float32)
    gwhh2 = np.zeros((NP, 3, NP), np.float32)
    for p in range(NP):
        sp = p // NV
        i = p % NV
        for gate in range(3):
            row = gwih_s[sp][gate * NV + i]
            for gdev in range(NP):
                if fam[gdev] == sp:
                    gwih2[gdev, gate, p] = row[perm[gdev] % NV]
            gwhh2[sp * NV:(sp + 1) * NV, gate, p] = \
                gwhh_s[sp][gate * NV + i]
    gwih2 = gwih2.astype(BF)
    gwhh2 = gwhh2.astype(BF)
    biasr2 = np.zeros((NP, 1), np.float32)
    biasz2 = np.zeros((NP, 1), np.float32)
    biasn2 = np.zeros((NP, 1), np.float32)
    bhhn2 = np.zeros((NP, 1), np.float32)
    for s in range(2):
        sl = slice(s * NV, (s + 1) * NV)
        biasr2[sl, 0] = gbih_s[s][:NV] + gbhh_s[s][:NV]
        biasz2[sl, 0] = gbih_s[s][NV:2 * NV] + gbhh_s[s][NV:2 * NV]
        biasn2[sl, 0] = gbih_s[s][2 * NV:]
        bhhn2[sl, 0] = gbhh_s[s][2 * NV:]
    i80 = np.eye(NP, dtype=np.float32).astype(BF)

    vavo2 = np.zeros((NP, 2), np.float32)
    vavo2[:NV, 0] = np.asarray(inp["va"], np.float32)
    vavo2[NV:, 1] = np.asarray(inp["vo"], np.float32)
    vavo2 = vavo2.astype(BF)

    m0a = np.asarray(inp["m0_a"], np.float32)
    m0o = np.asarray(inp["m0_o"], np.float32)
    Us = [np.asarray(inp[n], np.float32) for n in ("Ua", "Va", "Uo", "Vo")]
    mvec = [m0a, m0o, m0o, m0a]
    um0p = np.zeros((128, NJC, NP), np.float32)
    for gdev, (u, k) in enumerate(PAIRS):
        Um = Us[u][k] @ mvec[u]
        for hc in range(NJC):
            um0p[:, hc, gdev] = Um[hc * 128:(hc + 1) * 128]
    um0p = um0p.astype(BF)

    Ma = np.asarray(inp["Ma"], np.float32)
    Mo = np.asarray(inp["Mo"], np.float32)
    tm = [np.tanh(m0a @ Ma).astype(np.float32),
          np.tanh(m0o @ Mo).astype(np.float32)]
    tmv32 = np.zeros((128, 32), np.float32)
    for s in range(2):
        for b in range(BS):
            for hc in range(NJC):
                tmv32[:, s * 16 + b * 4 + hc] = \
                    tm[s][hc * 128:(hc + 1) * 128]

    laloT2 = np.zeros((NP, 10), np.float32)
    laloT2[:NV, :5] = np.asarray(inp["la_W"], np.float32).T
    laloT2[NV:, 5:] = np.asarray(inp["lo_W"], np.float32).T
    laloT2 = laloT2.astype(BF)
    lalob2 = np.zeros((10, 1), np.float32)
    lalob2[:5, 0] = np.asarray(inp["la_b"], np.float32)
    lalob2[5:, 0] = np.asarray(inp["lo_b"], np.float32)
    onesg = np.zeros((10, 2), np.float32)
    onesg[:5, 0] = 1.0
    onesg[5:, 1] = 1.0
    onesg = onesg.astype(BF)

    shared = dict(wihT=wihT, whhT=whhT, biasc=biasc, bhhn=bhhn,
                  i128=i128, um0p=um0p, i80=i80, gwih2=gwih2,
                  gwhh2=gwhh2, biasr2=biasr2, biasz2=biasz2,
                  biasn2=biasn2, bhhn2=bhhn2, vavo2=vavo2,
                  tmv32=tmv32, laloT2=laloT2, lalob2=lalob2,
                  onesg=onesg)

    in_maps = []
    for c in range(NCORE):
        t0g = c * CHUNK - HALO_M
        xcm = np.zeros((BS, TM, KIN * 128), np.float32)
        lo = max(0, -t0g)
        hi = min(TM, T - t0g)
        xcm[:, lo:hi, :] = xpad[:, t0g + lo:t0g + hi, :]
        xT = np.ascontiguousarray(
            xcm.transpose(2, 0, 1).reshape(KIN, 128, BS, TM)).astype(BF)
        hmaskM = np.ones((128, BS, 128), np.float32)
        smaskSv = np.ones((NP, BS, 128), np.float32)
        if c == 0:
            hmaskM[:, :, :HALO_M] = 0.0
            smaskSv[:, :, :HALO_S] = 0.0
        tmlocv = np.zeros((2, BS, CHUNK), np.float32)
        w0, w1 = c * CHUNK, (c + 1) * CHUNK
        n_valid = max(0, min(seq, w1) - w0)
        tmlocv[:, :, :n_valid] = 1.0
        ustat = np.zeros((PSH, NJC, 128, NJC, 128), np.float32)
        for i in range(PSH):
            u, k = PAIRS[c * PSH + i]
            ustat[i] = Us[u][k].reshape(NJC, 128, NJC, 128).transpose(
                2, 3, 0, 1)
        m = dict(shared)
        m.update(xT=xT, hmaskM=hmaskM, smaskS=smaskSv, tmloc=tmlocv,
                 ustat=ustat.astype(BF))
        in_maps.append(m)
    return in_maps


def kernel(**inputs):
    debug = bool(int(os.environ.get("CMLA_DEBUG", "0")))
    key = ("prog", debug, ITERS_MAIN, ITERS_SMALL)
    if key not in _CACHE:
        _CACHE[key] = build_program(debug=debug)
    nc = _CACHE[key]
    in_maps = _prep_inputs(inputs)
    res = run_bass_kernel_spmd(
        nc, in_maps, list(range(NCORE)),
        trace=bool(int(os.environ.get("CMLA_TRACE", "0"))))
    _CACHE["last_results"] = res
    ya = np.zeros((BS, T, NC), np.float32)
    yo = np.zeros((BS, T, NC), np.float32)
    for c in range(NCORE):
        y = res.results[c]["y"]
        ya[:, c * CHUNK:(c + 1) * CHUNK, :] = y[:5].transpose(1, 2, 0)
        yo[:, c * CHUNK:(c + 1) * CHUNK, :] = y[5:].transpose(1, 2, 0)
    return ya, yo
